# revision 6
# baseline (speedup 1.0000x reference)
"""GNN message-passing (scatter-add) kernel for 8 Trainium2 NeuronCores.

Computes out = segment_sum(x[src], dst, num_segments=N) for
x [10000, 128] f32, edge_index [2, 320000] int64.

Strategy — dense count-matrix matmul (no gathers, no collectives):
  out[d] = sum_s A[s, d] * x[s]   with A[s, d] = #edges s->d.

  - Host computes A (np.bincount over (src, dst) pairs) and shards its
    columns: core c owns dst range [c*1280, (c+1)*1280). A entries are
    small ints, exact in fp8e4 (<=16); larger counts split into extra
    passes (never triggers for random graphs).
  - On device, out^T[f, d] = sum_k x_k^T-stationary @ A_k-moving: the
    contraction runs over 80 source-node chunks of 128 on the PE, with
    x chunk [128 s, 128 f] fp16 stationary and A chunk [128 s, 1280 d]
    fp8 moving (512|512|256 splits), accumulating f32 into 3 persistent
    PSUM banks. A streams from HBM k-major in 8-chunk groups,
    overlapping DMA with PE.
  - fp16 x keeps relative L2 error ~1e-4 (resid_var ~1e-8); per-edge
    exactness of A makes the matmul otherwise exact.
  - Host transposes/concats the 8 cores' out^T tiles back to [10000, 128].

Per-core traffic: A 13.1MB + x 2.6MB + out 0.65MB ~= 16.4MB.
"""

import sys

for _p in ("/opt/trn_rl_repo",):
    if _p not in sys.path:
        sys.path.append(_p)

import ml_dtypes
import numpy as np

import concourse.bacc as bacc
import concourse.mybir as mybir
import concourse.tile as tile
from concourse.bass_utils import run_bass_kernel_spmd

N_NODES = 10000
D_FEAT = 128
N_CORES = 8
P = 128
NPAD = 10240  # padded node count: 80 source chunks of 128
KCH = NPAD // P  # 80 source chunks
DCORE = NPAD // N_CORES  # 1280 dst columns per core
DTILES = [(0, 512), (512, 512), (1024, 256)]  # psum-bank-sized dst splits
KGRP = 10  # source chunks per A DMA group
FP8 = ml_dtypes.float8_e4m3
FP8_MAX_INT = 16  # largest integer exactly representable in e4m3

# test/profiling hooks
TRACE = False
TRACE_CORES = None
LAST_RESULT = None


def _build_program(n_passes: int):
    nc = bacc.Bacc(
        "TRN2", target_bir_lowering=False, debug=False, num_devices=N_CORES
    )
    xt_d = nc.dram_tensor(
        "xt", [P, KCH * D_FEAT], mybir.dt.float16, kind="ExternalInput"
    )
    a_ds = [
        nc.dram_tensor(
            f"a{i}", [P, KCH * DCORE], mybir.dt.float8e4, kind="ExternalInput"
        )
        for i in range(n_passes)
    ]
    o_d = nc.dram_tensor("o", [P, DCORE], mybir.dt.float32, kind="ExternalOutput")

    with tile.TileContext(nc) as tc:
        with (
            tc.tile_pool(name="const", bufs=1) as constp,
            tc.tile_pool(name="a", bufs=3) as ap_,
            tc.tile_pool(name="res", bufs=2) as resp,
            tc.tile_pool(name="ps", bufs=1, space="PSUM") as psp,
        ):
            xt_sb = constp.tile([P, KCH, D_FEAT], mybir.dt.float16, tag="xt")
            nc.sync.dma_start(out=xt_sb[:], in_=xt_d[:])

            pss = [
                psp.tile([P, w], mybir.dt.float32, tag=f"ps{t}", name=f"ps{t}")
                for t, (off, w) in enumerate(DTILES)
            ]
            n_k = n_passes * KCH
            ki = 0
            for ip in range(n_passes):
                a_v = a_ds[ip][:].rearrange(
                    "p (k d) -> p k d", k=KCH, d=DCORE
                )
                for g in range(KCH // KGRP):
                    a_sb = ap_.tile([P, KGRP, DCORE], mybir.dt.float8e4, tag="a")
                    nc.sync.dma_start(
                        out=a_sb[:], in_=a_v[:, g * KGRP : (g + 1) * KGRP, :]
                    )
                    for kk in range(KGRP):
                        k = g * KGRP + kk
                        for t, (off, w) in enumerate(DTILES):
                            nc.tensor.matmul(
                                pss[t][:],
                                xt_sb[:, k, :],
                                a_sb[:, kk, off : off + w],
                                start=(ki == 0),
                                stop=(ki == n_k - 1),
                            )
                        ki += 1
            for t, (off, w) in enumerate(DTILES):
                res = resp.tile([P, w], mybir.dt.float32, tag=f"res{t}")
                nc.vector.tensor_copy(res[:], pss[t][:])
                nc.sync.dma_start(out=o_d[:, off : off + w], in_=res[:])

    nc.compile()
    return nc


def _prepare(x: np.ndarray, edge_index: np.ndarray):
    src = np.asarray(edge_index[0], dtype=np.int64)
    dst = np.asarray(edge_index[1], dtype=np.int64)

    xf = np.asarray(x, dtype=np.float32)
    xt = np.zeros((P, KCH, D_FEAT), np.float16)
    # xt[p, k, :] = x[k*128 + p, :]
    xt[:, : N_NODES // P, :] = (
        xf[: (N_NODES // P) * P].reshape(N_NODES // P, P, D_FEAT).transpose(1, 0, 2)
    )
    rem = N_NODES - (N_NODES // P) * P
    if rem:
        xt[:rem, N_NODES // P, :] = xf[(N_NODES // P) * P :]
    xt = np.ascontiguousarray(xt.reshape(P, KCH * D_FEAT))

    in_maps = []
    n_passes = 1
    per_core_As = []
    for c in range(N_CORES):
        sel = (dst >= c * DCORE) & (dst < (c + 1) * DCORE)
        idx = src[sel] * DCORE + (dst[sel] - c * DCORE)
        cnt = np.bincount(idx, minlength=NPAD * DCORE).reshape(NPAD, DCORE)
        passes = []
        while True:
            part = np.minimum(cnt, FP8_MAX_INT)
            passes.append(
                np.ascontiguousarray(
                    part.astype(FP8)
                    .reshape(KCH, P, DCORE)
                    .transpose(1, 0, 2)
                    .reshape(P, KCH * DCORE)
                )
            )
            cnt = cnt - part
            if not cnt.any():
                break
        per_core_As.append(passes)
        n_passes = max(n_passes, len(passes))

    zeros_a = None
    for c in range(N_CORES):
        m = {"xt": xt}
        for i in range(n_passes):
            if i < len(per_core_As[c]):
                m[f"a{i}"] = per_core_As[c][i]
            else:
                if zeros_a is None:
                    zeros_a = np.zeros((P, KCH * DCORE), FP8)
                m[f"a{i}"] = zeros_a
        in_maps.append(m)
    return in_maps, n_passes


def kernel(x: np.ndarray, edge_index: np.ndarray) -> np.ndarray:
    global LAST_RESULT
    in_maps, n_passes = _prepare(x, edge_index)
    nc = _build_program(n_passes)
    res = run_bass_kernel_spmd(
        nc,
        in_maps,
        list(range(N_CORES)),
        trace=TRACE,
        trace_cores=TRACE_CORES if TRACE else None,
    )
    LAST_RESULT = res
    # o per core: [128 f, 1280 d] -> out[c*1280 + d, f]
    out = np.concatenate(
        [np.asarray(r["o"], np.float32).T for r in res.results], axis=0
    )
    return np.ascontiguousarray(out[:N_NODES])


if __name__ == "__main__":
    rng = np.random.default_rng(0)
    x = rng.standard_normal((N_NODES, D_FEAT), dtype=np.float32)
    edge_index = rng.integers(0, N_NODES, size=(2, 320000)).astype(np.int64)
    out = kernel(x, edge_index)
    ref = np.zeros((N_NODES, D_FEAT), np.float32)
    np.add.at(ref, edge_index[1], x[edge_index[0]])
    rel = np.linalg.norm(out - ref) / np.linalg.norm(ref)
    print("rel L2 err:", rel)
